# revision 25
# baseline (speedup 1.0000x reference)
"""Trainium2 Bass kernel for nn_KnowledgeGuidedTransform.

The module is linear end-to-end, so the edge gather/scatter-mean folds into a
tiny dense matrix and the two Linear layers fuse:

    out[b,t] = mask * ((S_hat @ X[b,t]) @ Wcomb + C2b) + W_org_b

with S_hat (O x L) the normalized edge-count matrix, Wcomb = W1.T @ W_org.T,
and C2b absorbing concept/relation/bias terms. Host precomputes the small
constants; the device streams lab_features through two matmul stages.

Sharding: batch across the 8 cores (2 batches each); constants replicated.

Device schedule per core (X = (8192, 256) rows, out z = (2048, 256)):
  8 groups, each = 16 (b,t) steps = 1024 X rows = 256 output rows:
   - 1 MB DMA -> xb (128 x 2048): chunk k holds X rows 128k+p
   - mm1 (fp32r): lhsT = SD (block-diag stacked S_hat.T, 128x32, stationary),
     rhs = X pair chunks of both half-groups side by side (128 x 512)
     -> psY (r 128 x (blk,f) 512), partition-offset per pair. Y lands in
     (r, f) layout.
  - psY -> yb (SBUF), 4 PE transposes (128x128) -> psT -> yt: Y.T chunks.
  - mm2 (fp32r): per half-group, lhsT = Y.T f-chunk, rhs = Wcomb f-chunk
     (128x256), accumulate into psZ (r 128 x (blk,g) 512).
  - DVE adds the folded constant; 256 KB DMA out.
"""

import numpy as np

B, T, L, F = 16, 64, 64, 256
O, G = 16, 256
N_CORES = 8
B_LOC = B // N_CORES            # 2 batches per core
R_LOC = B_LOC * T * O           # 2048 output rows per core
XROWS = B_LOC * T * L           # 8192 X rows per core
N_GRP = 8                       # groups of 2 r-blocks

_compiled = None


def _build_nc():
    import concourse.bass as bass
    import concourse.tile as tile
    from concourse import bacc, mybir

    nc = bacc.Bacc(
        "TRN2", target_bir_lowering=False, debug=False, num_devices=1
    )
    f32 = mybir.dt.float32
    f32r = mybir.dt.float32r
    bf16 = mybir.dt.bfloat16
    x = nc.dram_tensor("x", [XROWS, F], bf16, kind="ExternalInput").ap()
    sd = nc.dram_tensor("sd", [128, 2 * O], bf16, kind="ExternalInput").ap()
    cst = nc.dram_tensor("cst", [128, 4 * G + 129], f32r, kind="ExternalInput").ap()
    z = nc.dram_tensor("z", [R_LOC, G], f32, kind="ExternalOutput").ap()

    xr = x.rearrange("(m k p) f -> m p k f", m=N_GRP // 2, k=16, p=128)
    zr = z.rearrange("(m b p) g -> m p b g", m=N_GRP // 2, b=4, p=128)

    with tile.TileContext(nc) as tc:
        with (
            tc.tile_pool(name="consts", bufs=1) as cpool,
            tc.tile_pool(name="xb", bufs=3) as xpool,
            tc.tile_pool(name="yb", bufs=2) as ybpool,
            tc.tile_pool(name="yt", bufs=2) as ytpool,
            tc.tile_pool(name="zt", bufs=3) as zpool,
            tc.tile_pool(name="psy", bufs=2, space="PSUM") as pypool,
            tc.tile_pool(name="pst", bufs=2, space="PSUM") as ptpool,
            tc.tile_pool(name="psz", bufs=2, space="PSUM") as pzpool,
        ):
            sd_t = cpool.tile([128, 2 * O], bf16, tag="sd")
            nc.sync.dma_start(sd_t[:], sd[:])
            cst_t = cpool.tile([128, 4 * G + 129], f32r, tag="cst")
            nc.sync.dma_start(cst_t[:], cst[:])
            wc_t = cst_t[:, 0 : 2 * G]
            cf_t = cst_t[:, 2 * G : 4 * G].bitcast(f32)
            invd_t = cst_t[:, 4 * G : 4 * G + 1].bitcast(f32)
            id_t = cst_t[:, 4 * G + 1 :].bitcast(f32)

            for n in range(N_GRP):
                if n % 2 == 0:
                    xb = xpool.tile([128, 16 * F], bf16, tag="xb")
                    nc.sync.dma_start(
                        xb.rearrange("p (k f) -> p k f", k=16), xr[n // 2]
                    )
                xbv = xb[:, (n % 2) * 8 * F : (n % 2 + 1) * 8 * F].rearrange(
                    "p (b i f) -> p b i f", b=2, i=4, f=F)

                psy = pypool.tile([128, 512], f32, tag="psy")
                psyv = psy.rearrange("p (b f) -> p b f", b=2)
                for i in range(4):
                    nc.tensor.matmul(
                        psyv[32 * i : 32 * (i + 1), :, :],
                        sd_t[:],
                        xbv[:, :, i, :],
                        start=True,
                        stop=True,
                        tile_position=(0, 32 * i),
                    )
                yb = ybpool.tile([128, 512], f32, tag="yb")
                nc.vector.tensor_scalar_mul(yb[:], psy[:], invd_t)

                # psT layout: [f0 blkA | f0 blkB | f1 blkA | f1 blkB]
                pst = ptpool.tile([128, 512], f32, tag="pst")
                for c in range(2):
                    for b in range(2):
                        nc.tensor.transpose(
                            pst[:, 128 * (2 * c + b) : 128 * (2 * c + b + 1)],
                            yb[:, 256 * b + 128 * c : 256 * b + 128 * (c + 1)],
                            id_t,
                        )
                yt = ytpool.tile([128, 512], f32r, tag="yt")
                nc.vector.tensor_copy(yt[:], pst[:])

                psz = pzpool.tile([128, 512], f32, tag="psz")
                pszv = psz.rearrange("p (b g) -> p b g", b=2)
                for b in range(2):
                    nc.tensor.matmul(
                        pszv[:, b, :],
                        yt[:, 128 * b : 128 * (b + 1)],
                        wc_t[:, 0:G],
                        start=True,
                        stop=False,
                    )
                    nc.tensor.matmul(
                        pszv[:, b, :],
                        yt[:, 256 + 128 * b : 256 + 128 * (b + 1)],
                        wc_t[:, G : 2 * G],
                        start=False,
                        stop=True,
                    )
                if n % 2 == 0:
                    zt = zpool.tile([128, 1024], f32, tag="zt")
                h = n % 2
                nc.vector.tensor_add(zt[:, 512 * h : 512 * (h + 1)], psz[:], cf_t)
                if h == 1:
                    nc.sync.dma_start(
                        zr[n // 2], zt.rearrange("p (b g) -> p b g", b=4)
                    )

    nc.compile()
    return nc


def _get_nc():
    global _compiled
    if _compiled is None:
        _compiled = _build_nc()
    return _compiled


def _precompute(lab_features, time_mask, edge_lab, edge_org, edge_rel,
                num_organs, lab_concept_emb, relation_emb, W_lab_w, W_lab_b,
                D_w, W_org_w, W_org_b):
    X = np.asarray(lab_features, np.float32)
    el = np.asarray(edge_lab).astype(np.int64)
    eo = np.asarray(edge_org).astype(np.int64)
    er = np.asarray(edge_rel).astype(np.int64)
    lab_concept_emb = np.asarray(lab_concept_emb, np.float32)
    relation_emb = np.asarray(relation_emb, np.float32)
    W_lab_w = np.asarray(W_lab_w, np.float32)
    W_lab_b = np.asarray(W_lab_b, np.float32)
    D_w = np.asarray(D_w, np.float32)
    W_org_w = np.asarray(W_org_w, np.float32)
    W_org_b = np.asarray(W_org_b, np.float32)
    n_org = int(num_organs)

    counts = np.bincount(eo, minlength=n_org).astype(np.float32)
    denom = np.maximum(counts, 1.0)

    S = np.zeros((n_org, L), np.float32)
    np.add.at(S, (eo, el), 1.0)
    S_hat = S / denom[:, None]

    rel_msg = relation_emb[er] @ D_w.T                      # (E, F)
    rel_bias = np.zeros((n_org, F), np.float32)
    np.add.at(rel_bias, eo, rel_msg)
    rel_bias /= denom[:, None]

    W1 = W_lab_w[:, :F]                                     # (F, F)
    hconc_b = lab_concept_emb @ W_lab_w[:, F:].T + W_lab_b  # (L, F)
    Wcomb = W1.T @ W_org_w.T                                # (F, G)
    C2b = (S_hat @ hconc_b + rel_bias) @ W_org_w.T + W_org_b  # (O, G)

    import ml_dtypes
    SD = np.zeros((128, 2 * O), np.float32)
    SD[:L, :O] = S.T
    SD[L:, O:] = S.T
    SD = SD.astype(ml_dtypes.bfloat16)
    X = X.astype(ml_dtypes.bfloat16)
    invden = np.tile((1.0 / denom).astype(np.float32), 8).reshape(128, 1)
    WC = np.concatenate([Wcomb[:128, :], Wcomb[128:, :]], axis=1)  # (128, 2G)
    CF1 = np.tile(C2b, (128 // n_org, 1)).astype(np.float32)       # (128, G)
    CF = np.concatenate([CF1, CF1], axis=1)                        # (128, 2G)

    CST = np.concatenate(
        [WC, CF, invden, np.eye(128, dtype=np.float32)], axis=1)
    return X, SD, CST, counts, W_org_b


def kernel(lab_features, time_mask, edge_lab, edge_org, edge_rel, num_organs,
           lab_concept_emb, relation_emb, W_lab_w, W_lab_b, D_w, W_org_w,
           W_org_b, _trace=False, _trace_kwargs=None):
    from concourse.bass_utils import run_bass_kernel_spmd

    X, SD, CST, counts, b_org = _precompute(
        lab_features, time_mask, edge_lab, edge_org, edge_rel, num_organs,
        lab_concept_emb, relation_emb, W_lab_w, W_lab_b, D_w, W_org_w, W_org_b)

    nc = _get_nc()
    in_maps = []
    for i in range(N_CORES):
        xs = np.ascontiguousarray(
            X[B_LOC * i : B_LOC * (i + 1)].reshape(XROWS, F))
        in_maps.append({"x": xs, "sd": SD, "cst": CST})

    res = run_bass_kernel_spmd(
        nc, in_maps, list(range(N_CORES)), trace=_trace,
        **(_trace_kwargs or {}))

    parts = [res.results[i]["z"].reshape(B_LOC, T, O, G) for i in range(N_CORES)]
    organ_states = np.concatenate(parts, axis=0)

    tm = np.asarray(time_mask).astype(bool)
    if not tm.all():
        m = tm.astype(np.float32)[:, :, None, None]
        organ_states = m * (organ_states - b_org) + b_org
    organ_states = organ_states.astype(np.float32)

    organ_mask = tm[:, :, None] & (counts > 0)[None, None, :]
    if _trace:
        return (organ_states, organ_mask), res
    return organ_states, organ_mask


# revision 26
# speedup vs baseline: 1.2792x; 1.2792x over previous
"""Trainium2 Bass kernel for nn_KnowledgeGuidedTransform.

The module is linear end-to-end, so the edge gather/scatter-mean folds into a
tiny dense matrix and the two Linear layers fuse:

    out[b,t] = mask * ((S_hat @ X[b,t]) @ Wcomb + C2b) + W_org_b

with S_hat (O x L) the normalized edge-count matrix, Wcomb = W1.T @ W_org.T,
and C2b absorbing concept/relation/bias terms. Host precomputes the small
constants; the device streams lab_features through two matmul stages.

Sharding: batch across the 8 cores (2 batches each); constants replicated.

Device schedule per core (X = (8192, 256) rows, out z = (2048, 256)):
  8 groups, each = 16 (b,t) steps = 1024 X rows = 256 output rows:
   - 1 MB DMA -> xb (128 x 2048): chunk k holds X rows 128k+p
   - mm1 (fp32r): lhsT = SD (block-diag stacked S_hat.T, 128x32, stationary),
     rhs = X pair chunks of both half-groups side by side (128 x 512)
     -> psY (r 128 x (blk,f) 512), partition-offset per pair. Y lands in
     (r, f) layout.
  - psY -> yb (SBUF), 4 PE transposes (128x128) -> psT -> yt: Y.T chunks.
  - mm2 (fp32r): per half-group, lhsT = Y.T f-chunk, rhs = Wcomb f-chunk
     (128x256), accumulate into psZ (r 128 x (blk,g) 512).
  - DVE adds the folded constant; 256 KB DMA out.
"""

import numpy as np

B, T, L, F = 16, 64, 64, 256
O, G = 16, 256
N_CORES = 8
B_LOC = B // N_CORES            # 2 batches per core
R_LOC = B_LOC * T * O           # 2048 output rows per core
XROWS = B_LOC * T * L           # 8192 X rows per core
N_GRP = 8                       # groups of 2 r-blocks

_compiled = None


def _build_nc():
    import concourse.bass as bass
    import concourse.tile as tile
    from concourse import bacc, mybir

    nc = bacc.Bacc(
        "TRN2", target_bir_lowering=False, debug=False, num_devices=1
    )
    f32 = mybir.dt.float32
    f32r = mybir.dt.float32r
    bf16 = mybir.dt.bfloat16
    x = nc.dram_tensor("x", [N_GRP // 2 * 128, 16 * F], bf16, kind="ExternalInput").ap()
    sd = nc.dram_tensor("sd", [128, 2 * O], bf16, kind="ExternalInput").ap()
    cst = nc.dram_tensor("cst", [128, 4 * G + 129], f32r, kind="ExternalInput").ap()
    z = nc.dram_tensor("z", [N_GRP // 2 * 128, 4 * G], f32, kind="ExternalOutput").ap()

    xr = x.rearrange("(m p) q -> m p q", m=N_GRP // 2)
    zr = z.rearrange("(m p) q -> m p q", m=N_GRP // 2)

    with tile.TileContext(nc) as tc:
        with (
            tc.tile_pool(name="consts", bufs=1) as cpool,
            tc.tile_pool(name="xb", bufs=3) as xpool,
            tc.tile_pool(name="yb", bufs=2) as ybpool,
            tc.tile_pool(name="yt", bufs=2) as ytpool,
            tc.tile_pool(name="zt", bufs=3) as zpool,
            tc.tile_pool(name="psy", bufs=2, space="PSUM") as pypool,
            tc.tile_pool(name="pst", bufs=2, space="PSUM") as ptpool,
            tc.tile_pool(name="psz", bufs=2, space="PSUM") as pzpool,
        ):
            sd_t = cpool.tile([128, 2 * O], bf16, tag="sd")
            nc.sync.dma_start(sd_t[:], sd[:])
            cst_t = cpool.tile([128, 4 * G + 129], f32r, tag="cst")
            nc.sync.dma_start(cst_t[:], cst[:])
            wc_t = cst_t[:, 0 : 2 * G]
            cf_t = cst_t[:, 2 * G : 4 * G].bitcast(f32)
            invd_t = cst_t[:, 4 * G : 4 * G + 1].bitcast(f32)
            id_t = cst_t[:, 4 * G + 1 :].bitcast(f32)

            for n in range(N_GRP):
                if n % 2 == 0:
                    xb = xpool.tile([128, 16 * F], bf16, tag="xb")
                    nc.sync.dma_start(xb[:], xr[n // 2])
                xbv = xb[:, (n % 2) * 8 * F : (n % 2 + 1) * 8 * F].rearrange(
                    "p (b i f) -> p b i f", b=2, i=4, f=F)

                psy = pypool.tile([128, 512], f32, tag="psy")
                psyv = psy.rearrange("p (b f) -> p b f", b=2)
                for i in range(4):
                    nc.tensor.matmul(
                        psyv[32 * i : 32 * (i + 1), :, :],
                        sd_t[:],
                        xbv[:, :, i, :],
                        start=True,
                        stop=True,
                        tile_position=(0, 32 * i),
                    )
                yb = ybpool.tile([128, 512], f32, tag="yb")
                nc.vector.tensor_scalar_mul(yb[:], psy[:], invd_t)

                # psT layout: [f0 blkA | f0 blkB | f1 blkA | f1 blkB]
                pst = ptpool.tile([128, 512], f32, tag="pst")
                for c in range(2):
                    for b in range(2):
                        nc.tensor.transpose(
                            pst[:, 128 * (2 * c + b) : 128 * (2 * c + b + 1)],
                            yb[:, 256 * b + 128 * c : 256 * b + 128 * (c + 1)],
                            id_t,
                        )
                yt = ytpool.tile([128, 512], f32r, tag="yt")
                nc.vector.tensor_copy(yt[:], pst[:])

                psz = pzpool.tile([128, 512], f32, tag="psz")
                pszv = psz.rearrange("p (b g) -> p b g", b=2)
                for b in range(2):
                    nc.tensor.matmul(
                        pszv[:, b, :],
                        yt[:, 128 * b : 128 * (b + 1)],
                        wc_t[:, 0:G],
                        start=True,
                        stop=False,
                    )
                    nc.tensor.matmul(
                        pszv[:, b, :],
                        yt[:, 256 + 128 * b : 256 + 128 * (b + 1)],
                        wc_t[:, G : 2 * G],
                        start=False,
                        stop=True,
                    )
                if n % 2 == 0:
                    zt = zpool.tile([128, 1024], f32, tag="zt")
                h = n % 2
                nc.vector.tensor_add(zt[:, 512 * h : 512 * (h + 1)], psz[:], cf_t)
                if h == 1:
                    nc.sync.dma_start(zr[n // 2], zt[:])

    nc.compile()
    return nc


def _get_nc():
    global _compiled
    if _compiled is None:
        _compiled = _build_nc()
    return _compiled


def _precompute(lab_features, time_mask, edge_lab, edge_org, edge_rel,
                num_organs, lab_concept_emb, relation_emb, W_lab_w, W_lab_b,
                D_w, W_org_w, W_org_b):
    X = np.asarray(lab_features, np.float32)
    el = np.asarray(edge_lab).astype(np.int64)
    eo = np.asarray(edge_org).astype(np.int64)
    er = np.asarray(edge_rel).astype(np.int64)
    lab_concept_emb = np.asarray(lab_concept_emb, np.float32)
    relation_emb = np.asarray(relation_emb, np.float32)
    W_lab_w = np.asarray(W_lab_w, np.float32)
    W_lab_b = np.asarray(W_lab_b, np.float32)
    D_w = np.asarray(D_w, np.float32)
    W_org_w = np.asarray(W_org_w, np.float32)
    W_org_b = np.asarray(W_org_b, np.float32)
    n_org = int(num_organs)

    counts = np.bincount(eo, minlength=n_org).astype(np.float32)
    denom = np.maximum(counts, 1.0)

    S = np.zeros((n_org, L), np.float32)
    np.add.at(S, (eo, el), 1.0)
    S_hat = S / denom[:, None]

    rel_msg = relation_emb[er] @ D_w.T                      # (E, F)
    rel_bias = np.zeros((n_org, F), np.float32)
    np.add.at(rel_bias, eo, rel_msg)
    rel_bias /= denom[:, None]

    W1 = W_lab_w[:, :F]                                     # (F, F)
    hconc_b = lab_concept_emb @ W_lab_w[:, F:].T + W_lab_b  # (L, F)
    Wcomb = W1.T @ W_org_w.T                                # (F, G)
    C2b = (S_hat @ hconc_b + rel_bias) @ W_org_w.T + W_org_b  # (O, G)

    import ml_dtypes
    SD = np.zeros((128, 2 * O), np.float32)
    SD[:L, :O] = S.T
    SD[L:, O:] = S.T
    SD = SD.astype(ml_dtypes.bfloat16)
    X = X.astype(ml_dtypes.bfloat16)
    invden = np.tile((1.0 / denom).astype(np.float32), 8).reshape(128, 1)
    WC = np.concatenate([Wcomb[:128, :], Wcomb[128:, :]], axis=1)  # (128, 2G)
    CF1 = np.tile(C2b, (128 // n_org, 1)).astype(np.float32)       # (128, G)
    CF = np.concatenate([CF1, CF1], axis=1)                        # (128, 2G)

    CST = np.concatenate(
        [WC, CF, invden, np.eye(128, dtype=np.float32)], axis=1)
    return X, SD, CST, counts, W_org_b


def kernel(lab_features, time_mask, edge_lab, edge_org, edge_rel, num_organs,
           lab_concept_emb, relation_emb, W_lab_w, W_lab_b, D_w, W_org_w,
           W_org_b, _trace=False, _trace_kwargs=None):
    from concourse.bass_utils import run_bass_kernel_spmd

    X, SD, CST, counts, b_org = _precompute(
        lab_features, time_mask, edge_lab, edge_org, edge_rel, num_organs,
        lab_concept_emb, relation_emb, W_lab_w, W_lab_b, D_w, W_org_w, W_org_b)

    nc = _get_nc()
    in_maps = []
    for i in range(N_CORES):
        xs = (X[B_LOC * i : B_LOC * (i + 1)].reshape(4, 16, 128, F)
              .transpose(0, 2, 1, 3).reshape(4 * 128, 16 * F))
        xs = np.ascontiguousarray(xs)
        in_maps.append({"x": xs, "sd": SD, "cst": CST})

    res = run_bass_kernel_spmd(
        nc, in_maps, list(range(N_CORES)), trace=_trace,
        **(_trace_kwargs or {}))

    parts = [res.results[i]["z"].reshape(4, 128, 4, G).transpose(0, 2, 1, 3)
             .reshape(B_LOC, T, O, G) for i in range(N_CORES)]
    organ_states = np.concatenate(parts, axis=0)

    tm = np.asarray(time_mask).astype(bool)
    if not tm.all():
        m = tm.astype(np.float32)[:, :, None, None]
        organ_states = m * (organ_states - b_org) + b_org
    organ_states = organ_states.astype(np.float32)

    organ_mask = tm[:, :, None] & (counts > 0)[None, None, :]
    if _trace:
        return (organ_states, organ_mask), res
    return organ_states, organ_mask
